# revision 20
# baseline (speedup 1.0000x reference)
"""Trainium2 Bass kernel for GAT-with-topology-bias (nn_Attntopo).

Math (per reference):
  h = x @ W                                  [N, F]
  e = leakyrelu(Wh1 + Wh2.T) * |W_ei| + (A + A^2 + A^3) * |W_si|
  attn = softmax(where(A > 0, e, -inf), axis=1)
  out = elu(attn @ h)

Distribution: row-shard the N x N work across 8 cores (rows_c = N/8 rows per
core).  Each core receives the full adj (bf16; 0/1 values are exact in bf16)
plus its row-slices, computes its block of rows, host concatenates.

Per-core device algorithm (all matmuls on the PE array, bf16 for the two big
N x N x N/8 products -- integer-valued, exact):
  ph0: h = x @ W (full), Wh1_c (own rows), B = broadcast(Wh2) tile
  ph1: AcT = (A_c).T via PE transposes                       [N, rows]
  ph2: PcT = A.T @ AcT = (A_c @ A).T   -> spilled to DRAM    [N, rows]
  ph3: for each column stripe: PQ = PcT.T @ (A + I) = (A^2 + A^3)_c rows,
       fused epilogue: scores -> masked online (flash) softmax -> attn @ h
  final: out = elu(o / l)
Diagonal zeroing of adj_w is a no-op post-masking (adj diag == 0), and the
(A+I) trick never touches the mask path (masks re-loaded from ar).
"""

import sys

sys.path.insert(0, "/opt/trn_rl_repo")

from contextlib import ExitStack

import numpy as np
import ml_dtypes

N = 6144
IN_F = 256
OUT_F = 64
NCORES = 8
ROWS = N // NCORES
SW = 384           # stripe width (A-column tiles per pass)
ALPHA = 0.2        # leaky relu slope
MASKV = -30000.0   # masked-score sentinel (exp() underflows to 0)

_BUILD_CACHE = {}


def build(n=N, rows=ROWS, sw=SW):
    key = (n, rows, sw)
    if key in _BUILD_CACHE:
        return _BUILD_CACHE[key]

    import concourse.bacc as bacc
    import concourse.tile as tile
    from concourse import mybir
    from concourse.masks import make_identity

    dt = mybir.dt
    f32 = dt.float32
    bf16 = dt.bfloat16
    f16 = dt.float16
    AF = mybir.ActivationFunctionType
    OP = mybir.AluOpType
    AX = mybir.AxisListType

    KT = n // 128          # 128-row tiles of A
    HK = KT // 2           # half-stripe k-tile count (DMA double-buffer unit)
    MT = rows // 128       # row tiles owned by this core
    NS = n // sw           # stripes
    M4 = sw // 128         # P^T row-tiles produced per phase-2 stripe
    KC = IN_F // 128       # input-feature chunks
    F = OUT_F

    nc = bacc.Bacc("TRN2", target_bir_lowering=False, debug=False,
                   num_devices=NCORES)

    x_d = nc.dram_tensor("x", [n, IN_F], f32, kind="ExternalInput")
    xr_d = nc.dram_tensor("xr", [rows, IN_F], f32, kind="ExternalInput")
    adj_d = nc.dram_tensor("adj", [n, n], bf16, kind="ExternalInput")
    ar_d = nc.dram_tensor("ar", [rows, n], bf16, kind="ExternalInput")
    w_d = nc.dram_tensor("W", [IN_F, F], f32, kind="ExternalInput")
    a_d = nc.dram_tensor("a", [2 * F, 1], f32, kind="ExternalInput")
    wsi_d = nc.dram_tensor("W_si", [1, 1], f32, kind="ExternalInput")
    wei_d = nc.dram_tensor("W_ei", [1, 1], f32, kind="ExternalInput")
    out_d = nc.dram_tensor("out", [rows, F], f32, kind="ExternalOutput")
    pct_d = nc.dram_tensor("pct", [n, rows], bf16)  # P_c.T spill

    with tile.TileContext(nc) as tc, ExitStack() as ctx:
        P = ctx.enter_context(tc.tile_pool(name="persist", bufs=1))
        id_f = P.tile([128, 128], f32, tag="id_f")
        make_identity(nc, id_f[:])
        id_b = P.tile([128, 128], bf16, tag="id_b")
        make_identity(nc, id_b[:])
        h_sb = P.tile([128, KT, F], f32, tag="h")
        h16 = P.tile([128, KT, F], f16, tag="h16")
        B_sb = P.tile([128, n], f32, tag="B")
        wh1w = P.tile([128, MT], f32, tag="wh1w")   # |W_ei| * Wh1 (own rows)
        wsi_bc = P.tile([128, 1], f32, tag="wsi")
        wei_bc = P.tile([128, 1], f32, tag="wei")
        o_st = P.tile([128, MT, F], f32, tag="o")
        l_st = P.tile([128, MT], f32, tag="l")
        m_st = P.tile([128, MT], f32, tag="m")
        nc.gpsimd.memset(o_st[:], 0.0)
        nc.gpsimd.memset(l_st[:], 0.0)
        nc.gpsimd.memset(m_st[:], MASKV)

        # ---------------- phase 0: h, Wh1_c, B, gate scalars -------------
        with tc.tile_pool(name="ph0", bufs=1) as p0, \
             tc.tile_pool(name="ph0w", bufs=3) as p0w, \
             tc.tile_pool(name="ph0ps", bufs=2, space="PSUM") as p0ps:
            w_sb = p0.tile([128, KC, F], f32, tag="w")
            for kc in range(KC):
                nc.sync.dma_start(w_sb[:, kc, :], w_d[kc * 128:(kc + 1) * 128, :])
            a1_sb = p0.tile([64, 1], f32, tag="a1")
            nc.sync.dma_start(a1_sb[:], a_d[0:F, :])
            a2_sb = p0.tile([64, 1], f32, tag="a2")
            nc.sync.dma_start(a2_sb[:], a_d[F:2 * F, :])
            ws = p0.tile([1, 1], f32, tag="ws")
            we = p0.tile([1, 1], f32, tag="we")
            nc.sync.dma_start(ws[:], wsi_d[:, :])
            nc.sync.dma_start(we[:], wei_d[:, :])
            wsa = p0.tile([1, 1], f32, tag="wsa")
            wea = p0.tile([1, 1], f32, tag="wea")
            nc.scalar.activation(wsa[:], ws[:], AF.Abs)
            nc.scalar.activation(wea[:], we[:], AF.Abs)
            nc.gpsimd.partition_broadcast(wsi_bc[:], wsa[:])
            nc.gpsimd.partition_broadcast(wei_bc[:], wea[:])

            xT = p0.tile([128, KC, n], f32, tag="xT")
            xrT = p0.tile([128, KC, rows], f32, tag="xrT")
            for r in range(KT):
                xt = p0w.tile([128, IN_F], f32, tag="xt")
                nc.sync.dma_start(xt[:], x_d[r * 128:(r + 1) * 128, :])
                for kc in range(KC):
                    tp = p0ps.tile([128, 128], f32, tag="tp0")
                    nc.tensor.transpose(tp[:], xt[:, kc * 128:(kc + 1) * 128], id_f[:])
                    nc.vector.tensor_copy(xT[:, kc, r * 128:(r + 1) * 128], tp[:])
            for r in range(MT):
                xt = p0w.tile([128, IN_F], f32, tag="xt")
                nc.sync.dma_start(xt[:], xr_d[r * 128:(r + 1) * 128, :])
                for kc in range(KC):
                    tp = p0ps.tile([128, 128], f32, tag="tp0")
                    nc.tensor.transpose(tp[:], xt[:, kc * 128:(kc + 1) * 128], id_f[:])
                    nc.vector.tensor_copy(xrT[:, kc, r * 128:(r + 1) * 128], tp[:])

            # h tiles + hT
            hT = p0.tile([64, n], f32, tag="hT")
            for r in range(KT):
                hp = p0ps.tile([128, F], f32, tag="hps")
                for kc in range(KC):
                    nc.tensor.matmul(hp[:], xT[:, kc, r * 128:(r + 1) * 128],
                                     w_sb[:, kc, :], start=(kc == 0),
                                     stop=(kc == KC - 1))
                nc.vector.tensor_copy(h_sb[:, r, :], hp[:])
                nc.vector.tensor_copy(h16[:, r, :], hp[:])
                tp = p0ps.tile([64, 128], f32, tag="tph")
                nc.tensor.transpose(tp[:], h_sb[:, r, :], id_f[:])
                hTs = p0w.tile([64, 128], f32, tag="hTs")
                nc.vector.tensor_copy(hTs[:], tp[:])
                nc.vector.tensor_copy(hT[:, r * 128:(r + 1) * 128], hTs[:])
            # Wh1 for own rows (h_c from xr), scaled by |W_ei|
            for m in range(MT):
                hp = p0ps.tile([128, F], f32, tag="hps")
                for kc in range(KC):
                    nc.tensor.matmul(hp[:], xrT[:, kc, m * 128:(m + 1) * 128],
                                     w_sb[:, kc, :], start=(kc == 0),
                                     stop=(kc == KC - 1))
                hcs = p0w.tile([128, F], f32, tag="hcs")
                nc.vector.tensor_copy(hcs[:], hp[:])
                tp = p0ps.tile([64, 128], f32, tag="tph")
                nc.tensor.transpose(tp[:], hcs[:], id_f[:])
                hct = p0w.tile([64, 128], f32, tag="hct")
                nc.vector.tensor_copy(hct[:], tp[:])
                wp = p0ps.tile([128, 1], f32, tag="wh1ps", bufs=1)
                nc.tensor.matmul(wp[:], hct[:], a1_sb[:], start=True, stop=True)
                nc.vector.tensor_copy(wh1w[:, m:m + 1], wp[:])
                nc.vector.tensor_scalar_mul(wh1w[:, m:m + 1], wh1w[:, m:m + 1],
                                            wei_bc[0:128, :])
            # Wh2 row vector, then broadcast into B
            w2r = p0.tile([1, n], f32, tag="w2r")
            for j in range(0, n, 512):
                wp = p0ps.tile([1, 512], f32, tag="w2ps", bufs=1)
                nc.tensor.matmul(wp[:], a2_sb[:], hT[:, j:j + 512],
                                 start=True, stop=True)
                nc.vector.tensor_copy(w2r[:, j:j + 512], wp[:])
            for j in range(0, n, 512):
                nc.gpsimd.partition_broadcast(B_sb[:, j:j + 512], w2r[:, j:j + 512])

        # ---------------- phase 1: AcT = (A_c).T --------------------------
        with tc.tile_pool(name="ph1a", bufs=1) as p1a:
            AcT = p1a.tile([128, KT, rows], bf16, tag="AcT")
            for k in range(KT):
                nc.sync.dma_start_transpose(
                    AcT[:, k, :], ar_d[:, k * 128:(k + 1) * 128])

            # ------------- phase 2: PcT = A.T @ AcT -> DRAM ---------------
            chunks = [(c, min(c + 512, rows)) for c in range(0, rows, 512)]
            with tc.tile_pool(name="ph2st", bufs=3) as p2s, \
                 tc.tile_pool(name="ph2o", bufs=3) as p2o, \
                 tc.tile_pool(name="ph2ps", bufs=3, space="PSUM") as p2ps:
                for s in range(NS):
                    halves = []
                    for hf in range(2):
                        st = p2s.tile([128, HK, sw], bf16, tag="st2")
                        for kk in range(HK):
                            k = hf * HK + kk
                            nc.sync.dma_start(
                                st[:, kk, :],
                                adj_d[k * 128:(k + 1) * 128, s * sw:(s + 1) * sw])
                        halves.append(st)
                    for m4 in range(M4):
                        pss = [p2ps.tile([128, c1 - c0], f32, tag=f"p2_{ci}",
                                         name=f"p2_{ci}")
                               for ci, (c0, c1) in enumerate(chunks)]
                        for k in range(KT):
                            lhs = halves[k // HK][:, k % HK,
                                                  m4 * 128:(m4 + 1) * 128]
                            for ci, (c0, c1) in enumerate(chunks):
                                nc.tensor.matmul(pss[ci][:], lhs,
                                                 AcT[:, k, c0:c1],
                                                 start=(k == 0),
                                                 stop=(k == KT - 1))
                        pout = p2o.tile([128, rows], bf16, tag="pout")
                        for ci, (c0, c1) in enumerate(chunks):
                            nc.vector.tensor_copy(pout[:, c0:c1], pss[ci][:])
                        row0 = (s * M4 + m4) * 128
                        nc.sync.dma_start(pct_d[row0:row0 + 128, :], pout[:])

        # ---------------- phase 3: PQ + fused masked flash softmax --------
        with tc.tile_pool(name="ph3p", bufs=1) as p3a, \
             tc.tile_pool(name="ph3st", bufs=3) as p3s, \
             tc.tile_pool(name="ph3mk", bufs=8) as p3m, \
             tc.tile_pool(name="ph3w", bufs=3) as p3w, \
             tc.tile_pool(name="ph3s", bufs=6) as p3ss, \
             tc.tile_pool(name="ph3ps", bufs=5, space="PSUM") as p3ps, \
             tc.tile_pool(name="ph3dl", bufs=2, space="PSUM") as p3dl:
            pct_sb = p3a.tile([128, KT, rows], bf16, tag="pct")
            for k in range(KT):
                nc.sync.dma_start(pct_sb[:, k, :], pct_d[k * 128:(k + 1) * 128, :])
            for s in range(NS):
                halves = []
                for hf in range(2):
                    st = p3s.tile([128, HK, sw], bf16, tag="st3")
                    for kk in range(HK):
                        k = hf * HK + kk
                        nc.sync.dma_start(
                            st[:, kk, :],
                            adj_d[k * 128:(k + 1) * 128, s * sw:(s + 1) * sw])
                    halves.append(st)
                # adj + I on the diagonal tiles of this stripe (matmul rhs only)
                for t in range(M4):
                    k = s * M4 + t
                    tgt = halves[k // HK][:, k % HK, t * 128:(t + 1) * 128]
                    nc.vector.tensor_tensor(tgt, tgt, id_b[:], op=OP.add)
                for m in range(MT):
                    mk = p3m.tile([128, sw], bf16, tag="mk")
                    nc.sync.dma_start(mk[:], ar_d[m * 128:(m + 1) * 128,
                                                  s * sw:(s + 1) * sw])
                    ps = p3ps.tile([128, sw], f32, tag="adjw")
                    for k in range(KT):
                        nc.tensor.matmul(ps[:],
                                         pct_sb[:, k, m * 128:(m + 1) * 128],
                                         halves[k // HK][:, k % HK, :],
                                         start=(k == 0), stop=(k == KT - 1))
                    # scores = |W_ei|*lrelu(Wh1_i + Wh2_j) + |W_si|*(A+A2+A3)
                    # lr = |W_ei| * lrelu(Wh2_j + Wh1_i)  (positive homogeneity:
                    # compute t = wei*B + wei*wh1, lrelu = max(t, alpha*t))
                    lr = p3w.tile([128, sw], f32, tag="lr")
                    t2 = p3w.tile([128, sw], f32, tag="t2")
                    nc.vector.tensor_scalar(lr[:], B_sb[:, s * sw:(s + 1) * sw],
                                            wei_bc[0:128, :], wh1w[:, m:m + 1],
                                            op0=OP.mult, op1=OP.add)
                    nc.vector.tensor_scalar_mul(t2[:], lr[:], ALPHA)
                    nc.vector.tensor_tensor(lr[:], lr[:], t2[:], op=OP.max)
                    u = p3w.tile([128, sw], f32, tag="u")
                    nc.vector.tensor_tensor(u[:], ps[:], mk[:], op=OP.add)
                    nc.vector.scalar_tensor_tensor(u[:], u[:], wsi_bc[0:128, :],
                                                   lr[:], op0=OP.mult, op1=OP.add)
                    # masked scores: sm = u*mk + (mk-1)*30000  (mk in {0,1};
                    # exact: u or MASKV, no rounding -- avoids CopyPredicated's
                    # integer-mask requirement)
                    sm = p3w.tile([128, sw], f32, tag="sm")
                    wm = p3w.tile([128, sw], f32, tag="wm")
                    nc.vector.tensor_scalar(wm[:], mk[:], -1.0, -MASKV,
                                            op0=OP.add, op1=OP.mult)
                    nc.vector.tensor_tensor(sm[:], u[:], mk[:], op=OP.mult)
                    nc.vector.tensor_tensor(sm[:], sm[:], wm[:], op=OP.add)
                    # online softmax update
                    bm = p3ss.tile([128, 1], f32, tag="bm")
                    nc.vector.tensor_reduce(bm[:], sm[:], axis=AX.X, op=OP.max)
                    g = p3ss.tile([128, 1], f32, tag="g")
                    nc.vector.tensor_tensor(g[:], bm[:], m_st[:, m:m + 1],
                                            op=OP.subtract)
                    nc.vector.tensor_scalar_max(g[:], g[:], 0.0)
                    sc = p3ss.tile([128, 1], f32, tag="sc")
                    nc.scalar.activation(sc[:], g[:], AF.Exp, scale=-1.0)
                    nc.vector.tensor_tensor(m_st[:, m:m + 1], m_st[:, m:m + 1],
                                            bm[:], op=OP.max)
                    negm = p3ss.tile([128, 1], f32, tag="negm")
                    nc.vector.tensor_scalar_mul(negm[:], m_st[:, m:m + 1], -1.0)
                    p = p3w.tile([128, sw], f16, tag="p")
                    rs = p3ss.tile([128, 1], f32, tag="rs")
                    nc.scalar.activation(p[:], sm[:], AF.Exp, bias=negm[:],
                                         accum_out=rs[:])
                    nc.vector.tensor_scalar_mul(l_st[:, m:m + 1], l_st[:, m:m + 1],
                                                sc[:])
                    nc.vector.tensor_tensor(l_st[:, m:m + 1], l_st[:, m:m + 1],
                                            rs[:], op=OP.add)
                    nc.vector.tensor_scalar_mul(o_st[:, m, :], o_st[:, m, :], sc[:])
                    dl = p3dl.tile([128, F], f32, tag="dl")
                    for t in range(M4):
                        pts = p3ss.tile([128, 128], f16, tag="pts")
                        nc.sync.dma_start_transpose(
                            pts[:], p[:, t * 128:(t + 1) * 128])
                        nc.tensor.matmul(dl[:], pts[:], h16[:, s * M4 + t, :],
                                         start=(t == 0), stop=(t == M4 - 1))
                    nc.vector.tensor_tensor(o_st[:, m, :], o_st[:, m, :], dl[:],
                                            op=OP.add)
            # --------- finalize: out = elu(o / l) -------------------------
            for m in range(MT):
                linv = p3ss.tile([128, 1], f32, tag="linv")
                nc.vector.reciprocal(linv[:], l_st[:, m:m + 1])
                hp = p3w.tile([128, F], f32, tag="hp")
                nc.vector.tensor_scalar_mul(hp[:], o_st[:, m, :], linv[:])
                mn = p3w.tile([128, F], f32, tag="mn")
                nc.vector.tensor_scalar_min(mn[:], hp[:], 0.0)
                ex = p3w.tile([128, F], f32, tag="ex")
                nc.scalar.activation(ex[:], mn[:], AF.Exp)
                nc.vector.tensor_scalar_add(ex[:], ex[:], -1.0)
                ot = p3w.tile([128, F], f32, tag="ot")
                nc.vector.tensor_tensor(ot[:], hp[:], ex[:], op=OP.max)
                nc.sync.dma_start(out_d[m * 128:(m + 1) * 128, :], ot[:])

    nc.compile()
    _BUILD_CACHE[key] = nc
    return nc


def make_in_maps(x, adj, W, a, W_si, W_ei, n=N, rows=ROWS):
    adj_bf = np.asarray(adj).astype(ml_dtypes.bfloat16)
    x = np.ascontiguousarray(np.asarray(x, dtype=np.float32))
    in_maps = []
    ncores = n // rows
    for c in range(ncores):
        rs = slice(c * rows, (c + 1) * rows)
        in_maps.append({
            "x": x,
            "xr": np.ascontiguousarray(x[rs]),
            "adj": adj_bf,
            "ar": np.ascontiguousarray(adj_bf[rs]),
            "W": np.asarray(W, dtype=np.float32),
            "a": np.asarray(a, dtype=np.float32),
            "W_si": np.asarray(W_si, dtype=np.float32),
            "W_ei": np.asarray(W_ei, dtype=np.float32),
        })
    return in_maps


def _ensure_ntff_hook():
    """The agent image's antenv lacks axon_hooks; shim it so trace=True
    can reach the NTFF profiler in libaxon_pjrt.so."""
    import types

    try:
        from antenv.axon_hooks import get_axon_ntff_profile_hook  # noqa: F401
        return
    except ImportError:
        pass
    import antenv

    mod = types.ModuleType("antenv.axon_hooks")
    mod._hook = None

    def set_axon_ntff_profile_hook(h):
        mod._hook = h

    def get_axon_ntff_profile_hook():
        return mod._hook

    mod.set_axon_ntff_profile_hook = set_axon_ntff_profile_hook
    mod.get_axon_ntff_profile_hook = get_axon_ntff_profile_hook
    sys.modules["antenv.axon_hooks"] = mod
    antenv.axon_hooks = mod
    try:
        if "/root/.axon_site" not in sys.path:
            sys.path.append("/root/.axon_site")
        from trn_agent_boot.trn_boot import _ntff_profile_via_ctypes

        mod._hook = _ntff_profile_via_ctypes("/opt/axon/libaxon_pjrt.so")
    except Exception:
        pass


def run(x, adj, W, a, W_si, W_ei, trace=False):
    from concourse.bass_utils import run_bass_kernel_spmd

    if trace:
        _ensure_ntff_hook()

    nc = build()
    in_maps = make_in_maps(x, adj, W, a, W_si, W_ei)
    res = run_bass_kernel_spmd(nc, in_maps, core_ids=list(range(NCORES)),
                               trace=trace)
    out = np.concatenate([np.asarray(res.results[c]["out"])
                          for c in range(NCORES)], axis=0)
    return out.astype(np.float32), res


def kernel(x, adj, W, a, W_si, W_ei):
    out, _ = run(x, adj, W, a, W_si, W_ei, trace=False)
    return out


# revision 32
# speedup vs baseline: 1.6496x; 1.6496x over previous
"""Trainium2 Bass kernel for GAT-with-topology-bias (nn_Attntopo).

Math (per reference):
  h = x @ W                                  [N, F]
  e = leakyrelu(Wh1 + Wh2.T) * |W_ei| + (A + A^2 + A^3) * |W_si|
  attn = softmax(where(A > 0, e, -inf), axis=1)
  out = elu(attn @ h)

Distribution: row-shard the N x N work across 8 cores (rows_c = N/8 rows per
core).  Each core receives the full adj (bf16; 0/1 values are exact in bf16)
plus its row-slices, computes its block of rows, host concatenates.

Per-core device algorithm (all matmuls on the PE array, bf16 for the two big
N x N x N/8 products -- integer-valued, exact):
  ph0: h = x @ W (full), Wh1_c (own rows), B = broadcast(Wh2) tile
  ph1: AcT = (A_c).T via PE transposes                       [N, rows]
  ph2: PcT = A.T @ AcT = (A_c @ A).T   -> spilled to DRAM    [N, rows]
  ph3: for each column stripe: PQ = PcT.T @ (A + I) = (A^2 + A^3)_c rows,
       fused epilogue: scores -> masked online (flash) softmax -> attn @ h
  final: out = elu(o / l)
Diagonal zeroing of adj_w is a no-op post-masking (adj diag == 0), and the
(A+I) trick never touches the mask path (masks re-loaded from ar).
"""

import sys

sys.path.insert(0, "/opt/trn_rl_repo")

from contextlib import ExitStack

import numpy as np
import ml_dtypes

N = 6144
IN_F = 256
OUT_F = 64
NCORES = 8
ROWS = N // NCORES
SW = 384           # stripe width (A-column tiles per pass)
ALPHA = 0.2        # leaky relu slope
MASKV = -30000.0   # masked-score sentinel (exp() underflows to 0)

_BUILD_CACHE = {}


USE_FP8 = True


def build(n=N, rows=ROWS, sw=SW, fp8=None):
    if fp8 is None:
        fp8 = USE_FP8
    key = (n, rows, sw, fp8)
    if key in _BUILD_CACHE:
        return _BUILD_CACHE[key]

    import concourse.bacc as bacc
    import concourse.tile as tile
    from concourse import mybir
    from concourse.masks import make_identity

    dt = mybir.dt
    f32 = dt.float32
    bf16 = dt.bfloat16
    f16 = dt.float16
    f8 = dt.float8e4
    adt = f8 if fp8 else bf16          # dtype of adj/AcT/pct matmul operands
    DR = mybir.MatmulPerfMode.DoubleRow if fp8 else None
    AF = mybir.ActivationFunctionType
    OP = mybir.AluOpType
    AX = mybir.AxisListType

    KT = n // 128          # 128-row tiles of A
    HK = KT // 2           # half-stripe k-tile count (DMA double-buffer unit)
    MT = rows // 128       # row tiles owned by this core
    NS = n // sw           # stripes
    M4 = sw // 128         # P^T row-tiles produced per phase-2 stripe
    KC = IN_F // 128       # input-feature chunks
    F = OUT_F

    nc = bacc.Bacc("TRN2", target_bir_lowering=False, debug=False,
                   num_devices=NCORES)

    x_d = nc.dram_tensor("x", [n, IN_F], f32, kind="ExternalInput")
    xr_d = nc.dram_tensor("xr", [rows, IN_F], f32, kind="ExternalInput")
    adj_d = nc.dram_tensor("adj", [n, n], adt, kind="ExternalInput")
    ar_d = nc.dram_tensor("ar", [rows, n], bf16, kind="ExternalInput")
    w_d = nc.dram_tensor("W", [IN_F, F], f32, kind="ExternalInput")
    a_d = nc.dram_tensor("a", [2 * F, 1], f32, kind="ExternalInput")
    wsi_d = nc.dram_tensor("W_si", [1, 1], f32, kind="ExternalInput")
    wei_d = nc.dram_tensor("W_ei", [1, 1], f32, kind="ExternalInput")
    out_d = nc.dram_tensor("out", [rows, F], f32, kind="ExternalOutput")
    pct_d = nc.dram_tensor("pct", [n, rows], adt)  # P_c.T spill

    with tile.TileContext(nc) as tc, ExitStack() as ctx:
        P = ctx.enter_context(tc.tile_pool(name="persist", bufs=1))
        id_f = P.tile([128, 128], f32, tag="id_f")
        make_identity(nc, id_f[:])
        id_b = P.tile([128, 128], bf16, tag="id_b")
        make_identity(nc, id_b[:])
        id_h = P.tile([128, 128], f16, tag="id_h")
        make_identity(nc, id_h[:])
        h_sb = P.tile([128, KT, F], f32, tag="h")
        h16 = P.tile([128, KT, F], f16, tag="h16")
        B_sb = P.tile([128, n], f32, tag="B")
        wh1w = P.tile([128, MT], f32, tag="wh1w")   # |W_ei| * Wh1 (own rows)
        wsi_bc = P.tile([128, 1], f32, tag="wsi")
        wei_bc = P.tile([128, 1], f32, tag="wei")
        o_st = P.tile([128, MT, F], f32, tag="o")
        l_st = P.tile([128, MT], f32, tag="l")
        m_st = P.tile([128, MT], f32, tag="m")
        nc.gpsimd.memset(o_st[:], 0.0)
        nc.gpsimd.memset(l_st[:], 0.0)
        nc.gpsimd.memset(m_st[:], MASKV)

        # ---------------- phase 0: h, Wh1_c, B, gate scalars -------------
        with tc.tile_pool(name="ph0", bufs=1) as p0, \
             tc.tile_pool(name="ph0w", bufs=3) as p0w, \
             tc.tile_pool(name="ph0ps", bufs=2, space="PSUM") as p0ps:
            w_sb = p0.tile([128, KC, F], f32, tag="w")
            for kc in range(KC):
                nc.sync.dma_start(w_sb[:, kc, :], w_d[kc * 128:(kc + 1) * 128, :])
            a1_sb = p0.tile([64, 1], f32, tag="a1")
            nc.sync.dma_start(a1_sb[:], a_d[0:F, :])
            a2_sb = p0.tile([64, 1], f32, tag="a2")
            nc.sync.dma_start(a2_sb[:], a_d[F:2 * F, :])
            ws = p0.tile([1, 1], f32, tag="ws")
            we = p0.tile([1, 1], f32, tag="we")
            nc.sync.dma_start(ws[:], wsi_d[:, :])
            nc.sync.dma_start(we[:], wei_d[:, :])
            wsa = p0.tile([1, 1], f32, tag="wsa")
            wea = p0.tile([1, 1], f32, tag="wea")
            nc.scalar.activation(wsa[:], ws[:], AF.Abs)
            nc.scalar.activation(wea[:], we[:], AF.Abs)
            nc.gpsimd.partition_broadcast(wsi_bc[:], wsa[:])
            nc.gpsimd.partition_broadcast(wei_bc[:], wea[:])

            xT = p0.tile([128, KC, n], f32, tag="xT")
            xrT = p0.tile([128, KC, rows], f32, tag="xrT")
            for r in range(KT):
                xt = p0w.tile([128, IN_F], f32, tag="xt")
                nc.sync.dma_start(xt[:], x_d[r * 128:(r + 1) * 128, :])
                for kc in range(KC):
                    tp = p0ps.tile([128, 128], f32, tag="tp0")
                    nc.tensor.transpose(tp[:], xt[:, kc * 128:(kc + 1) * 128], id_f[:])
                    nc.vector.tensor_copy(xT[:, kc, r * 128:(r + 1) * 128], tp[:])
            for r in range(MT):
                xt = p0w.tile([128, IN_F], f32, tag="xt")
                nc.sync.dma_start(xt[:], xr_d[r * 128:(r + 1) * 128, :])
                for kc in range(KC):
                    tp = p0ps.tile([128, 128], f32, tag="tp0")
                    nc.tensor.transpose(tp[:], xt[:, kc * 128:(kc + 1) * 128], id_f[:])
                    nc.vector.tensor_copy(xrT[:, kc, r * 128:(r + 1) * 128], tp[:])

            # h tiles + hT
            hT = p0.tile([64, n], f32, tag="hT")
            for r in range(KT):
                hp = p0ps.tile([128, F], f32, tag="hps")
                for kc in range(KC):
                    nc.tensor.matmul(hp[:], xT[:, kc, r * 128:(r + 1) * 128],
                                     w_sb[:, kc, :], start=(kc == 0),
                                     stop=(kc == KC - 1))
                nc.vector.tensor_copy(h_sb[:, r, :], hp[:])
                nc.vector.tensor_copy(h16[:, r, :], hp[:])
                tp = p0ps.tile([64, 128], f32, tag="tph")
                nc.tensor.transpose(tp[:], h_sb[:, r, :], id_f[:])
                hTs = p0w.tile([64, 128], f32, tag="hTs")
                nc.vector.tensor_copy(hTs[:], tp[:])
                nc.vector.tensor_copy(hT[:, r * 128:(r + 1) * 128], hTs[:])
            # Wh1 for own rows (h_c from xr), scaled by |W_ei|
            for m in range(MT):
                hp = p0ps.tile([128, F], f32, tag="hps")
                for kc in range(KC):
                    nc.tensor.matmul(hp[:], xrT[:, kc, m * 128:(m + 1) * 128],
                                     w_sb[:, kc, :], start=(kc == 0),
                                     stop=(kc == KC - 1))
                hcs = p0w.tile([128, F], f32, tag="hcs")
                nc.vector.tensor_copy(hcs[:], hp[:])
                tp = p0ps.tile([64, 128], f32, tag="tph")
                nc.tensor.transpose(tp[:], hcs[:], id_f[:])
                hct = p0w.tile([64, 128], f32, tag="hct")
                nc.vector.tensor_copy(hct[:], tp[:])
                wp = p0ps.tile([128, 1], f32, tag="wh1ps", bufs=1)
                nc.tensor.matmul(wp[:], hct[:], a1_sb[:], start=True, stop=True)
                nc.vector.tensor_copy(wh1w[:, m:m + 1], wp[:])
                nc.vector.tensor_scalar_mul(wh1w[:, m:m + 1], wh1w[:, m:m + 1],
                                            wei_bc[0:128, :])
            # Wh2 row vector, then broadcast into B
            w2r = p0.tile([1, n], f32, tag="w2r")
            for j in range(0, n, 512):
                wp = p0ps.tile([1, 512], f32, tag="w2ps", bufs=1)
                nc.tensor.matmul(wp[:], a2_sb[:], hT[:, j:j + 512],
                                 start=True, stop=True)
                nc.vector.tensor_copy(w2r[:, j:j + 512], wp[:])
            for j in range(0, n, 512):
                nc.gpsimd.partition_broadcast(B_sb[:, j:j + 512], w2r[:, j:j + 512])

        # ---------------- phase 1: AcT = (A_c).T --------------------------
        with tc.tile_pool(name="ph1a", bufs=1) as p1a:
            AcT = p1a.tile([128, KT, rows], adt, tag="AcT")
            if fp8:
                id_8 = P.tile([128, 128], f8, tag="id_8")
                nc.vector.tensor_copy(id_8[:], id_b[:])
                with tc.tile_pool(name="ph1s", bufs=3) as p1s:
                    for k in range(KT):
                        stg = p1s.tile([128, rows], bf16, tag="stg")
                        nc.sync.dma_start_transpose(
                            stg[:], ar_d[:, k * 128:(k + 1) * 128])
                        nc.vector.tensor_copy(AcT[:, k, :], stg[:])
            else:
                id_8 = id_b
                for k in range(KT):
                    nc.sync.dma_start_transpose(
                        AcT[:, k, :], ar_d[:, k * 128:(k + 1) * 128])

            # ------------- phase 2: PcT = A.T @ AcT -> DRAM ---------------
            chunks = [(c, min(c + 512, rows)) for c in range(0, rows, 512)]
            with tc.tile_pool(name="ph2st", bufs=3) as p2s, \
                 tc.tile_pool(name="ph2o", bufs=3) as p2o, \
                 tc.tile_pool(name="ph2ps", bufs=3, space="PSUM") as p2ps:
                for s in range(NS):
                    halves = []
                    for hf in range(2):
                        st = p2s.tile([128, HK, sw], adt, tag="st2")
                        for kk in range(HK):
                            k = hf * HK + kk
                            nc.sync.dma_start(
                                st[:, kk, :],
                                adj_d[k * 128:(k + 1) * 128, s * sw:(s + 1) * sw])
                        halves.append(st)
                    for m4 in range(M4):
                        pss = [p2ps.tile([128, c1 - c0], f32, tag=f"p2_{ci}",
                                         name=f"p2_{ci}")
                               for ci, (c0, c1) in enumerate(chunks)]
                        if fp8:
                            for t in range(KT // 2):
                                k = 2 * t
                                lhs = halves[k // HK][:, k % HK:k % HK + 2,
                                                      m4 * 128:(m4 + 1) * 128]
                                for ci, (c0, c1) in enumerate(chunks):
                                    nc.tensor.matmul(pss[ci][:], lhs,
                                                     AcT[:, k:k + 2, c0:c1],
                                                     start=(t == 0),
                                                     stop=(t == KT // 2 - 1),
                                                     perf_mode=DR)
                        else:
                            for k in range(KT):
                                lhs = halves[k // HK][:, k % HK,
                                                      m4 * 128:(m4 + 1) * 128]
                                for ci, (c0, c1) in enumerate(chunks):
                                    nc.tensor.matmul(pss[ci][:], lhs,
                                                     AcT[:, k, c0:c1],
                                                     start=(k == 0),
                                                     stop=(k == KT - 1))
                        pout = p2o.tile([128, rows], adt, tag="pout")
                        for ci, (c0, c1) in enumerate(chunks):
                            nc.vector.tensor_copy(pout[:, c0:c1], pss[ci][:])
                        row0 = (s * M4 + m4) * 128
                        nc.sync.dma_start(pct_d[row0:row0 + 128, :], pout[:])

        # ---------------- phase 3: PQ + fused masked flash softmax --------
        with tc.tile_pool(name="ph3p", bufs=1) as p3a, \
             tc.tile_pool(name="ph3st", bufs=3) as p3s, \
             tc.tile_pool(name="ph3mk", bufs=8) as p3m, \
             tc.tile_pool(name="ph3w", bufs=3) as p3w, \
             tc.tile_pool(name="ph3s", bufs=6) as p3ss, \
             tc.tile_pool(name="ph3ps", bufs=4, space="PSUM") as p3ps, \
             tc.tile_pool(name="ph3tp", bufs=2, space="PSUM") as p3tp, \
             tc.tile_pool(name="ph3dl", bufs=2, space="PSUM") as p3dl:
            pct_sb = p3a.tile([128, KT, rows], adt, tag="pct")
            for k in range(KT):
                nc.sync.dma_start(pct_sb[:, k, :], pct_d[k * 128:(k + 1) * 128, :])
            for s in range(NS):
                halves = []
                for hf in range(2):
                    st = p3s.tile([128, HK, sw], adt, tag="st3")
                    for kk in range(HK):
                        k = hf * HK + kk
                        nc.sync.dma_start(
                            st[:, kk, :],
                            adj_d[k * 128:(k + 1) * 128, s * sw:(s + 1) * sw])
                    halves.append(st)
                # adj + I on the diagonal tiles of this stripe (matmul rhs only)
                for t in range(M4):
                    k = s * M4 + t
                    tgt = halves[k // HK][:, k % HK, t * 128:(t + 1) * 128]
                    nc.vector.tensor_tensor(tgt, tgt, id_8[:], op=OP.add)
                for m in range(MT):
                    mk = p3m.tile([128, sw], bf16, tag="mk")
                    nc.sync.dma_start(mk[:], ar_d[m * 128:(m + 1) * 128,
                                                  s * sw:(s + 1) * sw])
                    ps = p3ps.tile([128, sw], f32, tag="adjw")
                    if fp8:
                        for t in range(KT // 2):
                            k = 2 * t
                            nc.tensor.matmul(
                                ps[:],
                                pct_sb[:, k:k + 2, m * 128:(m + 1) * 128],
                                halves[k // HK][:, k % HK:k % HK + 2, :],
                                start=(t == 0), stop=(t == KT // 2 - 1),
                                perf_mode=DR)
                    else:
                        for k in range(KT):
                            nc.tensor.matmul(
                                ps[:],
                                pct_sb[:, k, m * 128:(m + 1) * 128],
                                halves[k // HK][:, k % HK, :],
                                start=(k == 0), stop=(k == KT - 1))
                    # scores = |W_ei|*lrelu(Wh1_i + Wh2_j) + |W_si|*(A+A2+A3)
                    # lr = |W_ei| * lrelu(Wh2_j + Wh1_i)  (positive homogeneity:
                    # compute t = wei*B + wei*wh1, lrelu = max(t, alpha*t))
                    lr = p3w.tile([128, sw], f32, tag="lr")
                    t2 = p3w.tile([128, sw], f32, tag="t2")
                    nc.vector.tensor_scalar(lr[:], B_sb[:, s * sw:(s + 1) * sw],
                                            wei_bc[0:128, :], wh1w[:, m:m + 1],
                                            op0=OP.mult, op1=OP.add)
                    nc.vector.tensor_scalar_mul(t2[:], lr[:], ALPHA)
                    nc.vector.tensor_tensor(lr[:], lr[:], t2[:], op=OP.max)
                    u = p3w.tile([128, sw], f32, tag="u")
                    nc.vector.tensor_tensor(u[:], ps[:], mk[:], op=OP.add)
                    nc.vector.scalar_tensor_tensor(u[:], u[:], wsi_bc[0:128, :],
                                                   lr[:], op0=OP.mult, op1=OP.add)
                    # masked scores: sm = u*mk + (mk-1)*30000  (mk in {0,1};
                    # exact: u or MASKV, no rounding -- avoids CopyPredicated's
                    # integer-mask requirement)
                    sm = p3w.tile([128, sw], f32, tag="sm")
                    wm = p3w.tile([128, sw], f32, tag="wm")
                    nc.vector.tensor_scalar(wm[:], mk[:], -1.0, -MASKV,
                                            op0=OP.add, op1=OP.mult)
                    nc.vector.tensor_tensor(sm[:], u[:], mk[:], op=OP.mult)
                    nc.vector.tensor_tensor(sm[:], sm[:], wm[:], op=OP.add)
                    # online softmax update
                    bm = p3ss.tile([128, 1], f32, tag="bm")
                    nc.vector.tensor_reduce(bm[:], sm[:], axis=AX.X, op=OP.max)
                    g = p3ss.tile([128, 1], f32, tag="g")
                    nc.vector.tensor_tensor(g[:], bm[:], m_st[:, m:m + 1],
                                            op=OP.subtract)
                    nc.vector.tensor_scalar_max(g[:], g[:], 0.0)
                    sc = p3ss.tile([128, 1], f32, tag="sc")
                    nc.scalar.activation(sc[:], g[:], AF.Exp, scale=-1.0)
                    nc.vector.tensor_tensor(m_st[:, m:m + 1], m_st[:, m:m + 1],
                                            bm[:], op=OP.max)
                    negm = p3ss.tile([128, 1], f32, tag="negm")
                    nc.vector.tensor_scalar_mul(negm[:], m_st[:, m:m + 1], -1.0)
                    p = p3w.tile([128, sw], f16, tag="p")
                    rs = p3ss.tile([128, 1], f32, tag="rs")
                    nc.scalar.activation(p[:], sm[:], AF.Exp, bias=negm[:],
                                         accum_out=rs[:])
                    nc.vector.tensor_scalar_mul(l_st[:, m:m + 1], l_st[:, m:m + 1],
                                                sc[:])
                    nc.vector.tensor_tensor(l_st[:, m:m + 1], l_st[:, m:m + 1],
                                            rs[:], op=OP.add)
                    nc.vector.tensor_scalar_mul(o_st[:, m, :], o_st[:, m, :], sc[:])
                    dl = p3dl.tile([128, F], f32, tag="dl")
                    for t in range(M4):
                        tp = p3tp.tile([128, 128], f16, tag="tp3")
                        nc.tensor.transpose(tp[:], p[:, t * 128:(t + 1) * 128],
                                            id_h[:])
                        pts = p3ss.tile([128, 128], f16, tag="pts")
                        nc.vector.tensor_copy(pts[:], tp[:])
                        nc.tensor.matmul(dl[:], pts[:], h16[:, s * M4 + t, :],
                                         start=(t == 0), stop=(t == M4 - 1))
                    nc.vector.tensor_tensor(o_st[:, m, :], o_st[:, m, :], dl[:],
                                            op=OP.add)
            # --------- finalize: out = elu(o / l) -------------------------
            for m in range(MT):
                linv = p3ss.tile([128, 1], f32, tag="linv")
                nc.vector.reciprocal(linv[:], l_st[:, m:m + 1])
                hp = p3w.tile([128, F], f32, tag="hp")
                nc.vector.tensor_scalar_mul(hp[:], o_st[:, m, :], linv[:])
                mn = p3w.tile([128, F], f32, tag="mn")
                nc.vector.tensor_scalar_min(mn[:], hp[:], 0.0)
                ex = p3w.tile([128, F], f32, tag="ex")
                nc.scalar.activation(ex[:], mn[:], AF.Exp)
                nc.vector.tensor_scalar_add(ex[:], ex[:], -1.0)
                ot = p3w.tile([128, F], f32, tag="ot")
                nc.vector.tensor_tensor(ot[:], hp[:], ex[:], op=OP.max)
                nc.sync.dma_start(out_d[m * 128:(m + 1) * 128, :], ot[:])

    nc.compile()
    _BUILD_CACHE[key] = nc
    return nc


def make_in_maps(x, adj, W, a, W_si, W_ei, n=N, rows=ROWS, fp8=None):
    if fp8 is None:
        fp8 = USE_FP8
    adj_bf = np.asarray(adj).astype(ml_dtypes.bfloat16)
    adj_mm = adj_bf.astype(ml_dtypes.float8_e4m3) if fp8 else adj_bf
    x = np.ascontiguousarray(np.asarray(x, dtype=np.float32))
    in_maps = []
    ncores = n // rows
    for c in range(ncores):
        rs = slice(c * rows, (c + 1) * rows)
        in_maps.append({
            "x": x,
            "xr": np.ascontiguousarray(x[rs]),
            "adj": adj_mm,
            "ar": np.ascontiguousarray(adj_bf[rs]),
            "W": np.asarray(W, dtype=np.float32),
            "a": np.asarray(a, dtype=np.float32),
            "W_si": np.asarray(W_si, dtype=np.float32),
            "W_ei": np.asarray(W_ei, dtype=np.float32),
        })
    return in_maps


def _ensure_ntff_hook():
    """The agent image's antenv lacks axon_hooks; shim it so trace=True
    can reach the NTFF profiler in libaxon_pjrt.so."""
    import types

    try:
        from antenv.axon_hooks import get_axon_ntff_profile_hook  # noqa: F401
        return
    except ImportError:
        pass
    import antenv

    mod = types.ModuleType("antenv.axon_hooks")
    mod._hook = None

    def set_axon_ntff_profile_hook(h):
        mod._hook = h

    def get_axon_ntff_profile_hook():
        return mod._hook

    mod.set_axon_ntff_profile_hook = set_axon_ntff_profile_hook
    mod.get_axon_ntff_profile_hook = get_axon_ntff_profile_hook
    sys.modules["antenv.axon_hooks"] = mod
    antenv.axon_hooks = mod
    try:
        if "/root/.axon_site" not in sys.path:
            sys.path.append("/root/.axon_site")
        from trn_agent_boot.trn_boot import _ntff_profile_via_ctypes

        mod._hook = _ntff_profile_via_ctypes("/opt/axon/libaxon_pjrt.so")
    except Exception:
        pass


def run(x, adj, W, a, W_si, W_ei, trace=False):
    from concourse.bass_utils import run_bass_kernel_spmd

    if trace:
        _ensure_ntff_hook()

    nc = build()
    in_maps = make_in_maps(x, adj, W, a, W_si, W_ei)
    res = run_bass_kernel_spmd(nc, in_maps, core_ids=list(range(NCORES)),
                               trace=trace)
    out = np.concatenate([np.asarray(res.results[c]["out"])
                          for c in range(NCORES)], axis=0)
    return out.astype(np.float32), res


def kernel(x, adj, W, a, W_si, W_ei):
    out, _ = run(x, adj, W, a, W_si, W_ei, trace=False)
    return out


# revision 37
# speedup vs baseline: 1.8981x; 1.1507x over previous
"""Trainium2 Bass kernel for GAT-with-topology-bias (nn_Attntopo).

Math (per reference):
  h = x @ W                                  [N, F]
  e = leakyrelu(Wh1 + Wh2.T) * |W_ei| + (A + A^2 + A^3) * |W_si|
  attn = softmax(where(A > 0, e, -inf), axis=1)
  out = elu(attn @ h)

Distribution: row-shard the N x N work across 8 cores (rows_c = N/8 rows per
core).  Each core receives the full adj (bf16; 0/1 values are exact in bf16)
plus its row-slices, computes its block of rows, host concatenates.

Per-core device algorithm (all matmuls on the PE array, bf16 for the two big
N x N x N/8 products -- integer-valued, exact):
  ph0: h = x @ W (full), Wh1_c (own rows), B = broadcast(Wh2) tile
  ph1: AcT = (A_c).T via PE transposes                       [N, rows]
  ph2: PcT = A.T @ AcT = (A_c @ A).T   -> spilled to DRAM    [N, rows]
  ph3: for each column stripe: PQ = PcT.T @ (A + I) = (A^2 + A^3)_c rows,
       fused epilogue: scores -> masked online (flash) softmax -> attn @ h
  final: out = elu(o / l)
Diagonal zeroing of adj_w is a no-op post-masking (adj diag == 0), and the
(A+I) trick never touches the mask path (masks re-loaded from ar).
"""

import sys

sys.path.insert(0, "/opt/trn_rl_repo")

from contextlib import ExitStack

import numpy as np
import ml_dtypes

N = 6144
IN_F = 256
OUT_F = 64
NCORES = 8
ROWS = N // NCORES
SW = 768           # stripe load width (two compute sub-blocks per stripe)
ALPHA = 0.2        # leaky relu slope
MASKV = -30000.0   # masked-score sentinel (exp() underflows to 0)

_BUILD_CACHE = {}


USE_FP8 = True


def build(n=N, rows=ROWS, sw=SW, fp8=None):
    if fp8 is None:
        fp8 = USE_FP8
    key = (n, rows, sw, fp8)
    if key in _BUILD_CACHE:
        return _BUILD_CACHE[key]

    import concourse.bacc as bacc
    import concourse.tile as tile
    from concourse import mybir
    from concourse.masks import make_identity

    dt = mybir.dt
    f32 = dt.float32
    bf16 = dt.bfloat16
    f16 = dt.float16
    f8 = dt.float8e4
    adt = f8 if fp8 else bf16          # dtype of adj/AcT/pct matmul operands
    DR = mybir.MatmulPerfMode.DoubleRow if fp8 else None
    AF = mybir.ActivationFunctionType
    OP = mybir.AluOpType
    AX = mybir.AxisListType

    KT = n // 128          # 128-row tiles of A
    HK = KT // 2           # half-stripe k-tile count (DMA double-buffer unit)
    MT = rows // 128       # row tiles owned by this core
    NS = n // sw           # stripes
    M4 = sw // 128         # P^T row-tiles produced per phase-2 stripe
    NSUB = 2               # phase-3 compute sub-blocks per stripe
    SUBW = sw // NSUB      # phase-3 matmul N (must be <= 512)
    MSUB = SUBW // 128
    KC = IN_F // 128       # input-feature chunks
    F = OUT_F

    nc = bacc.Bacc("TRN2", target_bir_lowering=False, debug=False,
                   num_devices=NCORES)

    x_d = nc.dram_tensor("x", [n, IN_F], f32, kind="ExternalInput")
    xr_d = nc.dram_tensor("xr", [rows, IN_F], f32, kind="ExternalInput")
    adj_d = nc.dram_tensor("adj", [n, n], adt, kind="ExternalInput")
    ar_d = nc.dram_tensor("ar", [rows, n], bf16, kind="ExternalInput")
    w_d = nc.dram_tensor("W", [IN_F, F], f32, kind="ExternalInput")
    a_d = nc.dram_tensor("a", [2 * F, 1], f32, kind="ExternalInput")
    wsi_d = nc.dram_tensor("W_si", [1, 1], f32, kind="ExternalInput")
    wei_d = nc.dram_tensor("W_ei", [1, 1], f32, kind="ExternalInput")
    out_d = nc.dram_tensor("out", [rows, F], f32, kind="ExternalOutput")
    pct_d = nc.dram_tensor("pct", [n, rows], adt)  # P_c.T spill

    with tile.TileContext(nc) as tc, ExitStack() as ctx:
        P = ctx.enter_context(tc.tile_pool(name="persist", bufs=1))
        id_f = P.tile([128, 128], f32, tag="id_f")
        make_identity(nc, id_f[:])
        id_b = P.tile([128, 128], bf16, tag="id_b")
        make_identity(nc, id_b[:])
        id_h = P.tile([128, 128], f16, tag="id_h")
        make_identity(nc, id_h[:])
        h_sb = P.tile([128, KT, F], f32, tag="h")
        h16 = P.tile([128, KT, F], f16, tag="h16")
        B_sb = P.tile([128, n], f32, tag="B")
        wh1w = P.tile([128, MT], f32, tag="wh1w")   # |W_ei| * Wh1 (own rows)
        wsi_bc = P.tile([128, 1], f32, tag="wsi")
        wei_bc = P.tile([128, 1], f32, tag="wei")
        o_st = P.tile([128, MT, F], f32, tag="o")
        l_st = P.tile([128, MT], f32, tag="l")
        m_st = P.tile([128, MT], f32, tag="m")
        nc.gpsimd.memset(o_st[:], 0.0)
        nc.gpsimd.memset(l_st[:], 0.0)
        nc.gpsimd.memset(m_st[:], MASKV)

        # ---------------- phase 0: h, Wh1_c, B, gate scalars -------------
        with tc.tile_pool(name="ph0", bufs=1) as p0, \
             tc.tile_pool(name="ph0w", bufs=3) as p0w, \
             tc.tile_pool(name="ph0ps", bufs=2, space="PSUM") as p0ps:
            w_sb = p0.tile([128, KC, F], f32, tag="w")
            for kc in range(KC):
                nc.sync.dma_start(w_sb[:, kc, :], w_d[kc * 128:(kc + 1) * 128, :])
            a1_sb = p0.tile([64, 1], f32, tag="a1")
            nc.sync.dma_start(a1_sb[:], a_d[0:F, :])
            a2_sb = p0.tile([64, 1], f32, tag="a2")
            nc.sync.dma_start(a2_sb[:], a_d[F:2 * F, :])
            ws = p0.tile([1, 1], f32, tag="ws")
            we = p0.tile([1, 1], f32, tag="we")
            nc.sync.dma_start(ws[:], wsi_d[:, :])
            nc.sync.dma_start(we[:], wei_d[:, :])
            wsa = p0.tile([1, 1], f32, tag="wsa")
            wea = p0.tile([1, 1], f32, tag="wea")
            nc.scalar.activation(wsa[:], ws[:], AF.Abs)
            nc.scalar.activation(wea[:], we[:], AF.Abs)
            nc.gpsimd.partition_broadcast(wsi_bc[:], wsa[:])
            nc.gpsimd.partition_broadcast(wei_bc[:], wea[:])

            xT = p0.tile([128, KC, n], f32, tag="xT")
            xrT = p0.tile([128, KC, rows], f32, tag="xrT")
            for r in range(KT):
                xt = p0w.tile([128, IN_F], f32, tag="xt")
                nc.sync.dma_start(xt[:], x_d[r * 128:(r + 1) * 128, :])
                for kc in range(KC):
                    tp = p0ps.tile([128, 128], f32, tag="tp0")
                    nc.tensor.transpose(tp[:], xt[:, kc * 128:(kc + 1) * 128], id_f[:])
                    nc.vector.tensor_copy(xT[:, kc, r * 128:(r + 1) * 128], tp[:])
            for r in range(MT):
                xt = p0w.tile([128, IN_F], f32, tag="xt")
                nc.sync.dma_start(xt[:], xr_d[r * 128:(r + 1) * 128, :])
                for kc in range(KC):
                    tp = p0ps.tile([128, 128], f32, tag="tp0")
                    nc.tensor.transpose(tp[:], xt[:, kc * 128:(kc + 1) * 128], id_f[:])
                    nc.vector.tensor_copy(xrT[:, kc, r * 128:(r + 1) * 128], tp[:])

            # h tiles + hT
            hT = p0.tile([64, n], f32, tag="hT")
            for r in range(KT):
                hp = p0ps.tile([128, F], f32, tag="hps")
                for kc in range(KC):
                    nc.tensor.matmul(hp[:], xT[:, kc, r * 128:(r + 1) * 128],
                                     w_sb[:, kc, :], start=(kc == 0),
                                     stop=(kc == KC - 1))
                nc.vector.tensor_copy(h_sb[:, r, :], hp[:])
                nc.vector.tensor_copy(h16[:, r, :], hp[:])
                tp = p0ps.tile([64, 128], f32, tag="tph")
                nc.tensor.transpose(tp[:], h_sb[:, r, :], id_f[:])
                hTs = p0w.tile([64, 128], f32, tag="hTs")
                nc.vector.tensor_copy(hTs[:], tp[:])
                nc.vector.tensor_copy(hT[:, r * 128:(r + 1) * 128], hTs[:])
            # Wh1 for own rows (h_c from xr), scaled by |W_ei|
            for m in range(MT):
                hp = p0ps.tile([128, F], f32, tag="hps")
                for kc in range(KC):
                    nc.tensor.matmul(hp[:], xrT[:, kc, m * 128:(m + 1) * 128],
                                     w_sb[:, kc, :], start=(kc == 0),
                                     stop=(kc == KC - 1))
                hcs = p0w.tile([128, F], f32, tag="hcs")
                nc.vector.tensor_copy(hcs[:], hp[:])
                tp = p0ps.tile([64, 128], f32, tag="tph")
                nc.tensor.transpose(tp[:], hcs[:], id_f[:])
                hct = p0w.tile([64, 128], f32, tag="hct")
                nc.vector.tensor_copy(hct[:], tp[:])
                wp = p0ps.tile([128, 1], f32, tag="wh1ps", bufs=1)
                nc.tensor.matmul(wp[:], hct[:], a1_sb[:], start=True, stop=True)
                nc.vector.tensor_copy(wh1w[:, m:m + 1], wp[:])
                nc.vector.tensor_scalar_mul(wh1w[:, m:m + 1], wh1w[:, m:m + 1],
                                            wei_bc[0:128, :])
            # Wh2 row vector, then broadcast into B
            w2r = p0.tile([1, n], f32, tag="w2r")
            for j in range(0, n, 512):
                wp = p0ps.tile([1, 512], f32, tag="w2ps", bufs=1)
                nc.tensor.matmul(wp[:], a2_sb[:], hT[:, j:j + 512],
                                 start=True, stop=True)
                nc.vector.tensor_copy(w2r[:, j:j + 512], wp[:])
            for j in range(0, n, 512):
                nc.gpsimd.partition_broadcast(B_sb[:, j:j + 512], w2r[:, j:j + 512])

        # ---------------- phase 1: AcT = (A_c).T --------------------------
        with tc.tile_pool(name="ph1a", bufs=1) as p1a:
            AcT = p1a.tile([128, KT, rows], adt, tag="AcT")
            if fp8:
                id_8 = P.tile([128, 128], f8, tag="id_8")
                nc.vector.tensor_copy(id_8[:], id_b[:])
                with tc.tile_pool(name="ph1s", bufs=3) as p1s:
                    for k in range(KT):
                        stg = p1s.tile([128, rows], bf16, tag="stg")
                        nc.sync.dma_start_transpose(
                            stg[:], ar_d[:, k * 128:(k + 1) * 128])
                        nc.vector.tensor_copy(AcT[:, k, :], stg[:])
            else:
                id_8 = id_b
                for k in range(KT):
                    nc.sync.dma_start_transpose(
                        AcT[:, k, :], ar_d[:, k * 128:(k + 1) * 128])

            # ------------- phase 2: PcT = A.T @ AcT -> DRAM ---------------
            chunks = [(c, min(c + 512, rows)) for c in range(0, rows, 512)]
            with tc.tile_pool(name="ph2st", bufs=3) as p2s, \
                 tc.tile_pool(name="ph2o", bufs=3) as p2o, \
                 tc.tile_pool(name="ph2ps", bufs=3, space="PSUM") as p2ps:
                for s in range(NS):
                    halves = []
                    for hf in range(2):
                        st = p2s.tile([128, HK, sw], adt, tag="st2")
                        nc.sync.dma_start(
                            st[:],
                            adj_d[hf * HK * 128:(hf + 1) * HK * 128,
                                  s * sw:(s + 1) * sw]
                            .rearrange("(hk p) c -> p hk c", p=128))
                        halves.append(st)
                    for m4 in range(M4):
                        pss = [p2ps.tile([128, c1 - c0], f32, tag=f"p2_{ci}",
                                         name=f"p2_{ci}")
                               for ci, (c0, c1) in enumerate(chunks)]
                        if fp8:
                            for t in range(KT // 2):
                                k = 2 * t
                                lhs = halves[k // HK][:, k % HK:k % HK + 2,
                                                      m4 * 128:(m4 + 1) * 128]
                                for ci, (c0, c1) in enumerate(chunks):
                                    nc.tensor.matmul(pss[ci][:], lhs,
                                                     AcT[:, k:k + 2, c0:c1],
                                                     start=(t == 0),
                                                     stop=(t == KT // 2 - 1),
                                                     perf_mode=DR)
                        else:
                            for k in range(KT):
                                lhs = halves[k // HK][:, k % HK,
                                                      m4 * 128:(m4 + 1) * 128]
                                for ci, (c0, c1) in enumerate(chunks):
                                    nc.tensor.matmul(pss[ci][:], lhs,
                                                     AcT[:, k, c0:c1],
                                                     start=(k == 0),
                                                     stop=(k == KT - 1))
                        pout = p2o.tile([128, rows], adt, tag="pout")
                        for ci, (c0, c1) in enumerate(chunks):
                            nc.vector.tensor_copy(pout[:, c0:c1], pss[ci][:])
                        row0 = (s * M4 + m4) * 128
                        nc.sync.dma_start(pct_d[row0:row0 + 128, :], pout[:])

        # ---------------- phase 3: PQ + fused masked flash softmax --------
        with tc.tile_pool(name="ph3p", bufs=1) as p3a, \
             tc.tile_pool(name="ph3st", bufs=3) as p3s, \
             tc.tile_pool(name="ph3mk", bufs=2) as p3m, \
             tc.tile_pool(name="ph3w", bufs=3) as p3w, \
             tc.tile_pool(name="ph3s", bufs=6) as p3ss, \
             tc.tile_pool(name="ph3ps", bufs=4, space="PSUM") as p3ps, \
             tc.tile_pool(name="ph3tp", bufs=2, space="PSUM") as p3tp, \
             tc.tile_pool(name="ph3dl", bufs=2, space="PSUM") as p3dl:
            pct_sb = p3a.tile([128, KT, rows], adt, tag="pct")
            nc.sync.dma_start(pct_sb[:],
                              pct_d[:, :].rearrange("(k p) r -> p k r", p=128))
            for s in range(NS):
                halves = []
                for hf in range(2):
                    st = p3s.tile([128, HK, sw], adt, tag="st3")
                    nc.sync.dma_start(
                        st[:],
                        adj_d[hf * HK * 128:(hf + 1) * HK * 128,
                              s * sw:(s + 1) * sw]
                        .rearrange("(hk p) c -> p hk c", p=128))
                    halves.append(st)
                # adj + I on the diagonal tiles of this stripe (matmul rhs only)
                for t in range(M4):
                    k = s * M4 + t
                    tgt = halves[k // HK][:, k % HK, t * 128:(t + 1) * 128]
                    nc.vector.tensor_tensor(tgt, tgt, id_8[:], op=OP.add)
                mk_all = p3m.tile([128, MT, sw], bf16, tag="mk")
                nc.sync.dma_start(
                    mk_all[:],
                    ar_d[:, s * sw:(s + 1) * sw]
                    .rearrange("(m p) c -> p m c", p=128))
                for js in range(NSUB):
                  jc = js * SUBW             # col offset within stripe
                  j0 = s * sw + jc           # global col offset
                  for m in range(MT):
                    mk = mk_all[:, m, jc:jc + SUBW]
                    ps = p3ps.tile([128, SUBW], f32, tag="adjw")
                    if fp8:
                        for t in range(KT // 2):
                            k = 2 * t
                            nc.tensor.matmul(
                                ps[:],
                                pct_sb[:, k:k + 2, m * 128:(m + 1) * 128],
                                halves[k // HK][:, k % HK:k % HK + 2,
                                                jc:jc + SUBW],
                                start=(t == 0), stop=(t == KT // 2 - 1),
                                perf_mode=DR)
                    else:
                        for k in range(KT):
                            nc.tensor.matmul(
                                ps[:],
                                pct_sb[:, k, m * 128:(m + 1) * 128],
                                halves[k // HK][:, k % HK, jc:jc + SUBW],
                                start=(k == 0), stop=(k == KT - 1))
                    # lr = |W_ei| * lrelu(Wh2_j + Wh1_i)  (positive homogeneity:
                    # compute t = wei*B + wei*wh1, lrelu = max(t, alpha*t))
                    lr = p3w.tile([128, SUBW], f32, tag="lr")
                    t2 = p3w.tile([128, SUBW], f32, tag="t2")
                    nc.vector.tensor_scalar(lr[:], B_sb[:, j0:j0 + SUBW],
                                            wei_bc[0:128, :], wh1w[:, m:m + 1],
                                            op0=OP.mult, op1=OP.add)
                    nc.vector.tensor_scalar_mul(t2[:], lr[:], ALPHA)
                    nc.vector.tensor_tensor(lr[:], lr[:], t2[:], op=OP.max)
                    u = p3w.tile([128, SUBW], f32, tag="u")
                    nc.vector.tensor_tensor(u[:], ps[:], mk, op=OP.add)
                    nc.vector.scalar_tensor_tensor(u[:], u[:], wsi_bc[0:128, :],
                                                   lr[:], op0=OP.mult, op1=OP.add)
                    # masked scores: sm = u*mk + (mk-1)*30000  (mk in {0,1};
                    # exact: u or MASKV, no rounding)
                    sm = p3w.tile([128, SUBW], f32, tag="sm")
                    wm = p3w.tile([128, SUBW], f32, tag="wm")
                    nc.vector.tensor_scalar(wm[:], mk, -1.0, -MASKV,
                                            op0=OP.add, op1=OP.mult)
                    nc.vector.tensor_tensor(sm[:], u[:], mk, op=OP.mult)
                    nc.vector.tensor_tensor(sm[:], sm[:], wm[:], op=OP.add)
                    # online softmax update
                    bm = p3ss.tile([128, 1], f32, tag="bm")
                    nc.vector.tensor_reduce(bm[:], sm[:], axis=AX.X, op=OP.max)
                    g = p3ss.tile([128, 1], f32, tag="g")
                    nc.vector.tensor_tensor(g[:], bm[:], m_st[:, m:m + 1],
                                            op=OP.subtract)
                    nc.vector.tensor_scalar_max(g[:], g[:], 0.0)
                    sc = p3ss.tile([128, 1], f32, tag="sc")
                    nc.scalar.activation(sc[:], g[:], AF.Exp, scale=-1.0)
                    nc.vector.tensor_tensor(m_st[:, m:m + 1], m_st[:, m:m + 1],
                                            bm[:], op=OP.max)
                    negm = p3ss.tile([128, 1], f32, tag="negm")
                    nc.vector.tensor_scalar_mul(negm[:], m_st[:, m:m + 1], -1.0)
                    p = p3w.tile([128, SUBW], f16, tag="p")
                    rs = p3ss.tile([128, 1], f32, tag="rs")
                    nc.scalar.activation(p[:], sm[:], AF.Exp, bias=negm[:],
                                         accum_out=rs[:])
                    nc.vector.tensor_scalar_mul(l_st[:, m:m + 1], l_st[:, m:m + 1],
                                                sc[:])
                    nc.vector.tensor_tensor(l_st[:, m:m + 1], l_st[:, m:m + 1],
                                            rs[:], op=OP.add)
                    nc.vector.tensor_scalar_mul(o_st[:, m, :], o_st[:, m, :], sc[:])
                    dl = p3dl.tile([128, F], f32, tag="dl")
                    for t in range(MSUB):
                        tp = p3tp.tile([128, 128], f16, tag="tp3")
                        nc.tensor.transpose(tp[:], p[:, t * 128:(t + 1) * 128],
                                            id_h[:])
                        pts = p3ss.tile([128, 128], f16, tag="pts")
                        nc.vector.tensor_copy(pts[:], tp[:])
                        nc.tensor.matmul(dl[:], pts[:], h16[:, j0 // 128 + t, :],
                                         start=(t == 0), stop=(t == MSUB - 1))
                    nc.vector.tensor_tensor(o_st[:, m, :], o_st[:, m, :], dl[:],
                                            op=OP.add)
            # --------- finalize: out = elu(o / l) -------------------------
            for m in range(MT):
                linv = p3ss.tile([128, 1], f32, tag="linv")
                nc.vector.reciprocal(linv[:], l_st[:, m:m + 1])
                hp = p3w.tile([128, F], f32, tag="hp")
                nc.vector.tensor_scalar_mul(hp[:], o_st[:, m, :], linv[:])
                mn = p3w.tile([128, F], f32, tag="mn")
                nc.vector.tensor_scalar_min(mn[:], hp[:], 0.0)
                ex = p3w.tile([128, F], f32, tag="ex")
                nc.scalar.activation(ex[:], mn[:], AF.Exp)
                nc.vector.tensor_scalar_add(ex[:], ex[:], -1.0)
                ot = p3w.tile([128, F], f32, tag="ot")
                nc.vector.tensor_tensor(ot[:], hp[:], ex[:], op=OP.max)
                nc.sync.dma_start(out_d[m * 128:(m + 1) * 128, :], ot[:])

    nc.compile()
    _BUILD_CACHE[key] = nc
    return nc


def make_in_maps(x, adj, W, a, W_si, W_ei, n=N, rows=ROWS, fp8=None):
    if fp8 is None:
        fp8 = USE_FP8
    adj_bf = np.asarray(adj).astype(ml_dtypes.bfloat16)
    adj_mm = adj_bf.astype(ml_dtypes.float8_e4m3) if fp8 else adj_bf
    x = np.ascontiguousarray(np.asarray(x, dtype=np.float32))
    in_maps = []
    ncores = n // rows
    for c in range(ncores):
        rs = slice(c * rows, (c + 1) * rows)
        in_maps.append({
            "x": x,
            "xr": np.ascontiguousarray(x[rs]),
            "adj": adj_mm,
            "ar": np.ascontiguousarray(adj_bf[rs]),
            "W": np.asarray(W, dtype=np.float32),
            "a": np.asarray(a, dtype=np.float32),
            "W_si": np.asarray(W_si, dtype=np.float32),
            "W_ei": np.asarray(W_ei, dtype=np.float32),
        })
    return in_maps


def _ensure_ntff_hook():
    """The agent image's antenv lacks axon_hooks; shim it so trace=True
    can reach the NTFF profiler in libaxon_pjrt.so."""
    import types

    try:
        from antenv.axon_hooks import get_axon_ntff_profile_hook  # noqa: F401
        return
    except ImportError:
        pass
    import antenv

    mod = types.ModuleType("antenv.axon_hooks")
    mod._hook = None

    def set_axon_ntff_profile_hook(h):
        mod._hook = h

    def get_axon_ntff_profile_hook():
        return mod._hook

    mod.set_axon_ntff_profile_hook = set_axon_ntff_profile_hook
    mod.get_axon_ntff_profile_hook = get_axon_ntff_profile_hook
    sys.modules["antenv.axon_hooks"] = mod
    antenv.axon_hooks = mod
    try:
        if "/root/.axon_site" not in sys.path:
            sys.path.append("/root/.axon_site")
        from trn_agent_boot.trn_boot import _ntff_profile_via_ctypes

        mod._hook = _ntff_profile_via_ctypes("/opt/axon/libaxon_pjrt.so")
    except Exception:
        pass


def run(x, adj, W, a, W_si, W_ei, trace=False):
    from concourse.bass_utils import run_bass_kernel_spmd

    if trace:
        _ensure_ntff_hook()

    nc = build()
    in_maps = make_in_maps(x, adj, W, a, W_si, W_ei)
    res = run_bass_kernel_spmd(nc, in_maps, core_ids=list(range(NCORES)),
                               trace=trace)
    out = np.concatenate([np.asarray(res.results[c]["out"])
                          for c in range(NCORES)], axis=0)
    return out.astype(np.float32), res


def kernel(x, adj, W, a, W_si, W_ei):
    out, _ = run(x, adj, W, a, W_si, W_ei, trace=False)
    return out
